# revision 6
# baseline (speedup 1.0000x reference)
"""Trainium2 Bass kernel for CombinedLabelDistributionLoss.

Strategy (8 NeuronCores, SPMD, no collectives):
  - Batch-parallel: core c owns rows [8c, 8c+8) of predictions/targets,
    laid out in SBUF as [128 partitions x 256 free] (row b = 16 partitions).
  - Per-row standardization + Pearson partial sums on device; the five
    per-row sums are returned and finished on host (O(64) scalars).
  - The 140-bin DFT over the flattened standardized signal uses the
    angle-addition factorization  sin(theta*(256*o + r)) =
    sinO[o]cosI[r] + cosO[o]sinI[r]  with host-precomputed f32 tables,
    turning 73M transcendentals into two 128x128x280 PE matmuls per core
    plus elementwise combines. Per-core partial sin/cos sums over the
    owned slice of N are returned and reduced on host (8 x 140 floats).
  - All O(140) post-processing (softmax/KL/CE/argmax) happens on host in
    float32, mirroring the reference ops exactly.
"""

import math

import numpy as np

B, T = 64, 4096
NCORES = 8
RPC = B // NCORES          # rows per core = 8
P = 128                    # SBUF partitions
F = (RPC * T) // P         # free dim = 256
NBINS = 140
N = B * T                  # 262144
OUTER = F                  # outer stride in the n = OUTER*o + r factorization

_built = None              # cached (nc, const_inmaps_fn)


def _build_module():
    import concourse.bacc as bacc
    import concourse.bass as bass
    import concourse.mybir as mybir
    from concourse import tile

    dt = mybir.dt.float32
    AT = mybir.ActivationFunctionType
    ALU = mybir.AluOpType
    AX = mybir.AxisListType

    nc = bacc.Bacc(target_bir_lowering=False)

    x_d = nc.dram_tensor("x", [P, F], dt, kind="ExternalInput")
    y_d = nc.dram_tensor("y", [P, F], dt, kind="ExternalInput")
    win_d = nc.dram_tensor("win", [P, F], dt, kind="ExternalInput")
    outer_d = nc.dram_tensor("outer_tab", [P, 2 * NBINS], dt, kind="ExternalInput")
    inner_d = nc.dram_tensor("inner_tab", [P, 4 * NBINS], dt, kind="ExternalInput")
    rmask_d = nc.dram_tensor("rmask", [P, RPC], dt, kind="ExternalInput")
    rmaskT_d = nc.dram_tensor("rmaskT", [RPC, P], dt, kind="ExternalInput")
    pear_d = nc.dram_tensor("pear", [RPC, 5], dt, kind="ExternalOutput")
    dft_d = nc.dram_tensor("dft", [1, 8 * NBINS], dt, kind="ExternalOutput")

    with tile.TileContext(nc) as tc:
        with (
            tc.tile_pool(name="sb", bufs=1) as pool,
            tc.tile_pool(name="ps", bufs=2, space=bass.MemorySpace.PSUM) as psum,
        ):
            x = pool.tile([P, F], dt)
            nc.sync.dma_start(x[:], x_d[:])
            y = pool.tile([P, F], dt)
            nc.sync.dma_start(y[:], y_d[:])
            win = pool.tile([P, F], dt)
            nc.sync.dma_start(win[:], win_d[:])
            outer = pool.tile([P, 2 * NBINS], dt)
            nc.sync.dma_start(outer[:], outer_d[:])
            inner = pool.tile([P, 4 * NBINS], dt)
            nc.sync.dma_start(inner[:], inner_d[:])
            rmask = pool.tile([P, RPC], dt)
            nc.sync.dma_start(rmask[:], rmask_d[:])
            rmaskT = pool.tile([RPC, P], dt)
            nc.sync.dma_start(rmaskT[:], rmaskT_d[:])
            ones = pool.tile([P, 1], dt)
            nc.vector.memset(ones[:], 1.0)

            # ---- per-row mean of x ----
            csx = pool.tile([P, 1], dt)
            nc.vector.reduce_sum(out=csx[:], in_=x[:], axis=AX.X, op=ALU.add)
            rs_ps = psum.tile([RPC, 1], dt, tag="small")
            nc.tensor.matmul(rs_ps[:], rmask[:], csx[:])
            negmu = pool.tile([RPC, 1], dt)
            nc.scalar.mul(negmu[:], rs_ps[:], -1.0 / T)
            nmb_ps = psum.tile([P, 1], dt, tag="small")
            nc.tensor.matmul(nmb_ps[:], rmaskT[:], negmu[:])
            nmb = pool.tile([P, 1], dt)
            nc.scalar.copy(nmb[:], nmb_ps[:])

            # ---- per-row unbiased std: sq = (x - mu)^2, accum along free ----
            sq = pool.tile([P, F], dt)
            csq = pool.tile([P, 1], dt)
            nc.scalar.activation(
                sq[:], x[:], AT.Square, bias=nmb[:], scale=1.0, accum_out=csq[:]
            )
            ssq_ps = psum.tile([RPC, 1], dt, tag="small")
            nc.tensor.matmul(ssq_ps[:], rmask[:], csq[:])
            sd = pool.tile([RPC, 1], dt)
            nc.scalar.activation(sd[:], ssq_ps[:], AT.Sqrt, scale=1.0 / (T - 1))
            inv = pool.tile([RPC, 1], dt)
            nc.vector.reciprocal(inv[:], sd[:])
            invb_ps = psum.tile([P, 1], dt, tag="small")
            nc.tensor.matmul(invb_ps[:], rmaskT[:], inv[:])
            invb = pool.tile([P, 1], dt)
            nc.scalar.copy(invb[:], invb_ps[:])

            # ---- standardized p = (x - mu) * inv_sd, with col-sum accumulated ----
            stats = pool.tile([P, 5], dt)
            p = pool.tile([P, F], dt)
            nc.vector.tensor_scalar(
                out=p[:],
                in0=x[:],
                scalar1=nmb[:],
                scalar2=invb[:],
                op0=ALU.add,
                op1=ALU.mult,
            )
            nc.vector.reduce_sum(out=stats[:, 0:1], in_=p[:], axis=AX.X, op=ALU.add)

            # ---- Pearson partial sums ----
            nc.vector.reduce_sum(out=stats[:, 1:2], in_=y[:], axis=AX.X, op=ALU.add)
            py = pool.tile([P, F], dt)
            nc.vector.tensor_mul(py[:], p[:], y[:])
            nc.vector.reduce_sum(out=stats[:, 2:3], in_=py[:], axis=AX.X, op=ALU.add)
            nc.scalar.activation(sq[:], p[:], AT.Square, accum_out=stats[:, 3:4])
            sqy = pool.tile([P, F], dt)
            nc.scalar.activation(sqy[:], y[:], AT.Square, accum_out=stats[:, 4:5])
            pear_ps = psum.tile([RPC, 5], dt, tag="small")
            nc.tensor.matmul(pear_ps[:], rmask[:], stats[:])
            pear_sb = pool.tile([RPC, 5], dt)
            nc.scalar.copy(pear_sb[:], pear_ps[:])
            nc.sync.dma_start(pear_d[:], pear_sb[:])

            # ---- windowed signal ----
            sig = pool.tile([P, F], dt)
            nc.vector.tensor_mul(sig[:], p[:], win[:])

            # ---- DFT partials via angle-addition factorization ----
            # U_c[r, :] = [Us | Uc] = sig_chunk_c.T @ [sinO | cosO]
            prod = pool.tile([P, 8 * NBINS], dt)
            for c in range(2):
                U_ps = psum.tile([P, 2 * NBINS], dt, tag="dftU")
                nc.tensor.matmul(U_ps[:], sig[:, c * P:(c + 1) * P], outer[:])
                Us = U_ps[:, 0:NBINS]
                Uc = U_ps[:, NBINS:2 * NBINS]
                cosI = inner[:, (2 * c) * NBINS:(2 * c + 1) * NBINS]
                sinI = inner[:, (2 * c + 1) * NBINS:(2 * c + 2) * NBINS]
                # sin contributions: Us*cosI + Uc*sinI   (cols 0..559)
                nc.vector.tensor_mul(prod[:, (2 * c) * NBINS:(2 * c + 1) * NBINS], Us, cosI)
                nc.vector.tensor_mul(prod[:, (2 * c + 1) * NBINS:(2 * c + 2) * NBINS], Uc, sinI)
                # cos contributions: Uc*cosI - Us*sinI   (cols 560..1119, minus handled on host)
                nc.vector.tensor_mul(prod[:, (4 + 2 * c) * NBINS:(5 + 2 * c) * NBINS], Uc, cosI)
                nc.vector.tensor_mul(prod[:, (5 + 2 * c) * NBINS:(6 + 2 * c) * NBINS], Us, sinI)

            # partition-reduce prod via ones-matmul, 512 cols at a time
            red_sb = pool.tile([1, 8 * NBINS], dt)
            for i, (lo, hi) in enumerate(((0, 512), (512, 1024), (1024, 1120))):
                red_ps = psum.tile([1, hi - lo], dt, tag="dftred")
                nc.tensor.matmul(red_ps[:], ones[:], prod[:, lo:hi])
                nc.vector.tensor_copy(red_sb[:, lo:hi], red_ps[:])
            nc.sync.dma_start(dft_d[:], red_sb[:])

    nc.compile()
    return nc


def _tables(frame_rate: int):
    """Host-precomputed f32 constant tables (depend only on frame_rate)."""
    bpm = np.arange(40.0, 180.0, dtype=np.float32)
    k32 = (bpm / np.float32(60.0)) / (np.float32(frame_rate) / np.float32(N))
    theta = k32.astype(np.float64) * (2.0 * math.pi) / N   # [140]

    o = np.arange(NCORES * P, dtype=np.float64)            # global outer index
    ang_o = theta[None, :] * OUTER * o[:, None]            # [1024, 140]
    sinO = np.sin(ang_o).astype(np.float32)
    cosO = np.cos(ang_o).astype(np.float32)
    outer_tabs = [
        np.concatenate([sinO[c * P:(c + 1) * P], cosO[c * P:(c + 1) * P]], axis=1)
        for c in range(NCORES)
    ]                                                      # each [128, 280]

    r = np.arange(F, dtype=np.float64)
    ang_r = theta[None, :] * r[:, None]                    # [256, 140]
    sinI = np.sin(ang_r).astype(np.float32)
    cosI = np.cos(ang_r).astype(np.float32)
    inner_tab = np.concatenate(
        [cosI[0:P], sinI[0:P], cosI[P:2 * P], sinI[P:2 * P]], axis=1
    )                                                      # [128, 560]

    window = np.hanning(N).astype(np.float32).reshape(NCORES, P, F)

    rmask = np.zeros((P, RPC), dtype=np.float32)
    for b in range(RPC):
        rmask[b * (P // RPC):(b + 1) * (P // RPC), b] = 1.0
    rmaskT = np.ascontiguousarray(rmask.T)

    return outer_tabs, inner_tab, window, rmask, rmaskT


_tables_cache = {}


def kernel(predictions, targets, avg_hr, frame_rate, a, b):
    from concourse.bass_utils import run_bass_kernel_spmd

    global _built
    if _built is None:
        _built = _build_module()
    nc = _built

    avg_hr = int(avg_hr)
    frame_rate = int(frame_rate)
    a = int(a)
    b = int(b)

    if frame_rate not in _tables_cache:
        _tables_cache[frame_rate] = _tables(frame_rate)
    outer_tabs, inner_tab, window, rmask, rmaskT = _tables_cache[frame_rate]

    preds = np.ascontiguousarray(predictions, dtype=np.float32)
    targs = np.ascontiguousarray(targets, dtype=np.float32)

    in_maps = []
    for c in range(NCORES):
        in_maps.append({
            "x": preds[c * RPC:(c + 1) * RPC].reshape(P, F),
            "y": targs[c * RPC:(c + 1) * RPC].reshape(P, F),
            "win": window[c],
            "outer_tab": outer_tabs[c],
            "inner_tab": inner_tab,
            "rmask": rmask,
            "rmaskT": rmaskT,
        })

    res = run_bass_kernel_spmd(nc, in_maps, core_ids=list(range(NCORES)))
    results = res.results

    # ---- host finish: Pearson ----
    sums = np.concatenate([results[c]["pear"] for c in range(NCORES)], axis=0)
    sums = sums.astype(np.float32)                          # [64, 5]
    sum_x, sum_y, sum_xy, sum_x2, sum_y2 = (sums[:, i] for i in range(5))
    Nt = np.float32(T)
    pearson = (Nt * sum_xy - sum_x * sum_y) / np.sqrt(
        (Nt * sum_x2 - sum_x ** 2) * (Nt * sum_y2 - sum_y ** 2))
    loss_rppg = np.float32(np.mean(np.float32(1.0) - pearson, dtype=np.float32))

    # ---- host finish: spectrum ----
    sin_part = np.zeros(NBINS, dtype=np.float32)
    cos_part = np.zeros(NBINS, dtype=np.float32)
    for c in range(NCORES):
        d = results[c]["dft"][0].astype(np.float32)
        sin_part += d[0:140] + d[140:280] + d[280:420] + d[420:560]
        cos_part += d[560:700] - d[700:840] + d[840:980] - d[980:1120]

    ca = sin_part ** 2 + cos_part ** 2
    ca = (ca / np.sum(ca)).astype(np.float32)

    t_idx = avg_hr - 40
    i = np.arange(NBINS, dtype=np.float64)
    td = np.exp(-(i - t_idx) ** 2 / 2.0) / math.sqrt(2.0 * math.pi)
    td = np.maximum(td, 1e-15).astype(np.float32)

    m = np.max(ca)
    e = np.exp(ca - m)
    freq = (e / np.sum(e)).astype(np.float32)
    loss_kl = np.float32(np.sum(td * (np.log(td) - np.log(freq))) / np.float32(140.0))

    loss_ce = np.float32(np.log(np.sum(np.exp(ca - m))) + m - ca[t_idx])
    mae_hr = np.float32(abs(float(t_idx) - float(np.argmax(ca))))

    total = np.float32(a) * loss_rppg + np.float32(b) * (loss_ce + loss_kl)
    return (np.float32(total), np.float32(loss_rppg), np.float32(loss_kl),
            np.float32(loss_ce), np.float32(mae_hr))


# revision 9
# speedup vs baseline: 1.2487x; 1.2487x over previous
"""Trainium2 Bass kernel for CombinedLabelDistributionLoss.

Strategy (8 NeuronCores, SPMD, no collectives):
  - Batch-parallel: core c owns rows [8c, 8c+8) of predictions/targets,
    laid out in SBUF as [128 partitions x 256 free] (row b = 16 partitions).
  - Pearson is computed from RAW-data sums (sum x, y, xy, x^2, y^2) --
    standardization cancels algebraically -- finished on host (O(64)).
  - Per-row mean/std still computed on device (one-pass variance) to form
    the standardized, Hann-windowed signal for the DFT term.
  - The 140-bin DFT over the flattened standardized signal uses the
    angle-addition factorization  sin(theta*(256*o + r)) =
    sinO[o]cosI[r] + cosO[o]sinI[r]  with host-precomputed bf16 tables,
    turning 73M transcendentals into two bf16 128x128x280 PE matmuls per
    core plus elementwise combines; the partition reduction runs as
    PSUM-accumulated ones-matmuls. Per-core partial sin/cos sums are
    reduced on host (8 x 140 floats).
  - All O(140) post-processing (softmax/KL/CE/argmax) happens on host in
    float32, mirroring the reference ops exactly.
"""

import math

import numpy as np

B, T = 64, 4096
NCORES = 8
RPC = B // NCORES          # rows per core = 8
P = 128                    # SBUF partitions
F = (RPC * T) // P         # free dim = 256
NBINS = 140
N = B * T                  # 262144
OUTER = F                  # outer stride in the n = OUTER*o + r factorization

_built = None


def _build_module():
    import concourse.bacc as bacc
    import concourse.bass as bass
    import concourse.mybir as mybir
    from concourse import tile

    f32 = mybir.dt.float32
    bf16 = mybir.dt.bfloat16
    AT = mybir.ActivationFunctionType
    ALU = mybir.AluOpType
    AX = mybir.AxisListType

    nc = bacc.Bacc(target_bir_lowering=False)

    x_d = nc.dram_tensor("x", [P, F], f32, kind="ExternalInput")
    y_d = nc.dram_tensor("y", [P, F], f32, kind="ExternalInput")
    win_d = nc.dram_tensor("win", [P, F], f32, kind="ExternalInput")
    outer_d = nc.dram_tensor("outer_tab", [P, 2 * NBINS], bf16, kind="ExternalInput")
    inner_d = nc.dram_tensor("inner_tab", [P, 4 * NBINS], f32, kind="ExternalInput")
    rmask_d = nc.dram_tensor("rmask", [P, RPC], f32, kind="ExternalInput")
    rmaskT_d = nc.dram_tensor("rmaskT", [RPC, P], f32, kind="ExternalInput")
    pear_d = nc.dram_tensor("pear", [RPC, 5], f32, kind="ExternalOutput")
    dft_d = nc.dram_tensor("dft", [1, 4 * NBINS], f32, kind="ExternalOutput")

    with tile.TileContext(nc) as tc:
        with (
            tc.tile_pool(name="sb", bufs=1) as pool,
            tc.tile_pool(name="ps", bufs=2, space=bass.MemorySpace.PSUM) as psum,
            tc.tile_pool(name="ps1", bufs=1, space=bass.MemorySpace.PSUM) as psum1,
        ):
            # ---- ACT table prefetch (Square then Sqrt used later) ----
            dummy = pool.tile([1, 1], f32)
            nc.vector.memset(dummy[:], 1.0)
            dummy2 = pool.tile([1, 1], f32)
            nc.scalar.activation(dummy2[:], dummy[:], AT.Square)
            nc.scalar.activation(dummy2[:], dummy[:], AT.Sqrt)

            x = pool.tile([P, F], f32)
            nc.sync.dma_start(x[:], x_d[:])
            y = pool.tile([P, F], f32)
            nc.sync.dma_start(y[:], y_d[:])
            win = pool.tile([P, F], f32)
            nc.sync.dma_start(win[:], win_d[:])
            outer = pool.tile([P, 2 * NBINS], bf16)
            nc.sync.dma_start(outer[:], outer_d[:])
            inner = pool.tile([P, 4 * NBINS], f32)
            nc.sync.dma_start(inner[:], inner_d[:])
            rmask = pool.tile([P, RPC], f32)
            nc.sync.dma_start(rmask[:], rmask_d[:])
            rmaskT = pool.tile([RPC, P], f32)
            nc.sync.dma_start(rmaskT[:], rmaskT_d[:])
            ones_bf = pool.tile([P, 1], bf16)
            nc.vector.memset(ones_bf[:], 1.0)

            # ---- raw stats: [sx, sy, sxy, sx2, sy2] ----
            stats = pool.tile([P, 5], f32)
            nc.vector.reduce_sum(out=stats[:, 0:1], in_=x[:], axis=AX.X, op=ALU.add)
            nc.vector.reduce_sum(out=stats[:, 1:2], in_=y[:], axis=AX.X, op=ALU.add)
            xy = pool.tile([P, F], f32)
            nc.gpsimd.tensor_mul(xy[:], x[:], y[:])
            nc.vector.reduce_sum(out=stats[:, 2:3], in_=xy[:], axis=AX.X, op=ALU.add)
            sqx = pool.tile([P, F], f32)
            nc.scalar.activation(sqx[:], x[:], AT.Square, accum_out=stats[:, 3:4])
            sqy = pool.tile([P, F], f32)
            nc.scalar.activation(sqy[:], y[:], AT.Square, accum_out=stats[:, 4:5])

            pear_ps = psum.tile([RPC, 5], f32, tag="small")
            nc.tensor.matmul(pear_ps[:], rmask[:], stats[:])
            pear_sb = pool.tile([RPC, 5], f32)
            nc.scalar.copy(pear_sb[:], pear_ps[:])
            nc.sync.dma_start(pear_d[:], pear_sb[:])

            # ---- per-row -mean and 1/std from the raw sums ----
            mu = pool.tile([RPC, 1], f32)
            nc.scalar.mul(mu[:], pear_ps[:, 0:1], 1.0 / T)
            t2 = pool.tile([RPC, 1], f32)
            nc.vector.tensor_mul(t2[:], pear_ps[:, 0:1], mu[:])
            ssq = pool.tile([RPC, 1], f32)
            nc.vector.tensor_sub(ssq[:], pear_ps[:, 3:4], t2[:])
            packed = pool.tile([RPC, 2], f32)
            nc.scalar.mul(packed[:, 0:1], mu[:], -1.0)
            sd = pool.tile([RPC, 1], f32)
            nc.scalar.activation(sd[:], ssq[:], AT.Sqrt, scale=1.0 / (T - 1))
            nc.vector.reciprocal(packed[:, 1:2], sd[:])

            nb_ps = psum.tile([P, 2], f32, tag="small")
            nc.tensor.matmul(nb_ps[:], rmaskT[:], packed[:])
            nb = pool.tile([P, 2], f32)
            nc.scalar.copy(nb[:], nb_ps[:])

            # ---- standardized, windowed signal (bf16 for the PE) ----
            pstd = pool.tile([P, F], f32)
            nc.vector.tensor_scalar(
                out=pstd[:], in0=x[:], scalar1=nb[:, 0:1], scalar2=nb[:, 1:2],
                op0=ALU.add, op1=ALU.mult,
            )
            sig_bf = pool.tile([P, F], bf16)
            nc.vector.tensor_mul(sig_bf[:], pstd[:], win[:])

            # ---- DFT partials via angle-addition factorization ----
            red_sin = psum1.tile([1, 2 * NBINS], f32)
            red_cos = psum1.tile([1, 2 * NBINS], f32)
            for c in range(2):
                U_ps = psum.tile([P, 2 * NBINS], f32, tag="dftU")
                nc.tensor.matmul(U_ps[:], sig_bf[:, c * P:(c + 1) * P], outer[:])
                Us = U_ps[:, 0:NBINS]
                Uc = U_ps[:, NBINS:2 * NBINS]
                cosI = inner[:, (2 * c) * NBINS:(2 * c + 1) * NBINS]
                sinI = inner[:, (2 * c + 1) * NBINS:(2 * c + 2) * NBINS]
                # sin contribs [Us*cosI | Uc*sinI], cos contribs [Uc*cosI | Us*sinI]
                prod_s = pool.tile([P, 2 * NBINS], bf16, tag="prods")
                prod_c = pool.tile([P, 2 * NBINS], bf16, tag="prodc")
                nc.vector.tensor_mul(prod_s[:, 0:NBINS], Us, cosI)
                nc.vector.tensor_mul(prod_s[:, NBINS:2 * NBINS], Uc, sinI)
                nc.vector.tensor_mul(prod_c[:, 0:NBINS], Uc, cosI)
                nc.vector.tensor_mul(prod_c[:, NBINS:2 * NBINS], Us, sinI)
                # partition-reduce, accumulating both chunks into one PSUM tile
                nc.tensor.matmul(red_sin[:], ones_bf[:], prod_s[:],
                                 start=(c == 0), stop=(c == 1))
                nc.tensor.matmul(red_cos[:], ones_bf[:], prod_c[:],
                                 start=(c == 0), stop=(c == 1))

            red_sb = pool.tile([1, 4 * NBINS], f32)
            nc.scalar.copy(red_sb[:, 0:2 * NBINS], red_sin[:])
            nc.scalar.copy(red_sb[:, 2 * NBINS:4 * NBINS], red_cos[:])
            nc.sync.dma_start(dft_d[:], red_sb[:])

    nc.compile()
    return nc


def _tables(frame_rate: int):
    """Host-precomputed constant tables (depend only on frame_rate)."""
    import ml_dtypes

    nbf = ml_dtypes.bfloat16
    bpm = np.arange(40.0, 180.0, dtype=np.float32)
    k32 = (bpm / np.float32(60.0)) / (np.float32(frame_rate) / np.float32(N))
    theta = k32.astype(np.float64) * (2.0 * math.pi) / N   # [140]

    o = np.arange(NCORES * P, dtype=np.float64)            # global outer index
    ang_o = theta[None, :] * OUTER * o[:, None]            # [1024, 140]
    sinO = np.sin(ang_o).astype(nbf)
    cosO = np.cos(ang_o).astype(nbf)
    outer_tabs = [
        np.ascontiguousarray(np.concatenate(
            [sinO[c * P:(c + 1) * P], cosO[c * P:(c + 1) * P]], axis=1))
        for c in range(NCORES)
    ]                                                      # each [128, 280] bf16

    r = np.arange(F, dtype=np.float64)
    ang_r = theta[None, :] * r[:, None]                    # [256, 140]
    sinI = np.sin(ang_r).astype(np.float32)
    cosI = np.cos(ang_r).astype(np.float32)
    inner_tab = np.ascontiguousarray(np.concatenate(
        [cosI[0:P], sinI[0:P], cosI[P:2 * P], sinI[P:2 * P]], axis=1))

    window = np.hanning(N).astype(np.float32).reshape(NCORES, P, F)

    rmask = np.zeros((P, RPC), dtype=np.float32)
    for b in range(RPC):
        rmask[b * (P // RPC):(b + 1) * (P // RPC), b] = 1.0
    rmaskT = np.ascontiguousarray(rmask.T)

    return outer_tabs, inner_tab, window, rmask, rmaskT


_tables_cache = {}


def _make_in_maps(preds, targs, frame_rate):
    if frame_rate not in _tables_cache:
        _tables_cache[frame_rate] = _tables(frame_rate)
    outer_tabs, inner_tab, window, rmask, rmaskT = _tables_cache[frame_rate]
    in_maps = []
    for c in range(NCORES):
        in_maps.append({
            "x": preds[c * RPC:(c + 1) * RPC].reshape(P, F),
            "y": targs[c * RPC:(c + 1) * RPC].reshape(P, F),
            "win": window[c],
            "outer_tab": outer_tabs[c],
            "inner_tab": inner_tab,
            "rmask": rmask,
            "rmaskT": rmaskT,
        })
    return in_maps


def _finish(results, avg_hr, a, b):
    # ---- Pearson from raw sums ----
    sums = np.concatenate([results[c]["pear"] for c in range(NCORES)], axis=0)
    sums = sums.astype(np.float32)                          # [64, 5]
    sum_x, sum_y, sum_xy, sum_x2, sum_y2 = (sums[:, i] for i in range(5))
    Nt = np.float32(T)
    pearson = (Nt * sum_xy - sum_x * sum_y) / np.sqrt(
        (Nt * sum_x2 - sum_x ** 2) * (Nt * sum_y2 - sum_y ** 2))
    loss_rppg = np.float32(np.mean(np.float32(1.0) - pearson, dtype=np.float32))

    # ---- spectrum ----
    sin_part = np.zeros(NBINS, dtype=np.float32)
    cos_part = np.zeros(NBINS, dtype=np.float32)
    for c in range(NCORES):
        d = results[c]["dft"][0].astype(np.float32)
        sin_part += d[0:140] + d[140:280]
        cos_part += d[280:420] - d[420:560]

    ca = sin_part ** 2 + cos_part ** 2
    ca = (ca / np.sum(ca)).astype(np.float32)

    t_idx = avg_hr - 40
    i = np.arange(NBINS, dtype=np.float64)
    td = np.exp(-(i - t_idx) ** 2 / 2.0) / math.sqrt(2.0 * math.pi)
    td = np.maximum(td, 1e-15).astype(np.float32)

    m = np.max(ca)
    e = np.exp(ca - m)
    freq = (e / np.sum(e)).astype(np.float32)
    loss_kl = np.float32(np.sum(td * (np.log(td) - np.log(freq))) / np.float32(140.0))

    loss_ce = np.float32(np.log(np.sum(np.exp(ca - m))) + m - ca[t_idx])
    mae_hr = np.float32(abs(float(t_idx) - float(np.argmax(ca))))

    total = np.float32(a) * loss_rppg + np.float32(b) * (loss_ce + loss_kl)
    return (np.float32(total), np.float32(loss_rppg), np.float32(loss_kl),
            np.float32(loss_ce), np.float32(mae_hr))


def kernel(predictions, targets, avg_hr, frame_rate, a, b):
    from concourse.bass_utils import run_bass_kernel_spmd

    global _built
    if _built is None:
        _built = _build_module()

    preds = np.ascontiguousarray(predictions, dtype=np.float32)
    targs = np.ascontiguousarray(targets, dtype=np.float32)
    in_maps = _make_in_maps(preds, targs, int(frame_rate))
    res = run_bass_kernel_spmd(nc=_built, in_maps=in_maps,
                               core_ids=list(range(NCORES)))
    return _finish(res.results, int(avg_hr), int(a), int(b))


# revision 11
# speedup vs baseline: 1.5164x; 1.2144x over previous
"""Trainium2 Bass kernel for CombinedLabelDistributionLoss.

Strategy (8 NeuronCores, SPMD, no collectives):
  - Batch-parallel: core c owns rows [8c, 8c+8) of predictions/targets.
  - Pearson is computed from RAW-data sums (sum x, y, xy, x^2, y^2) on
    device -- standardization cancels algebraically -- finished on host.
  - The 140-bin DFT over the flattened standardized signal uses the
    angle-addition factorization  sin(theta*(256*o + r)) =
    sinO[o]cosI[r] + cosO[o]sinI[r].  The device DFTs the RAW windowed
    signal (x*hann), contracting over r first (so the row identity, which
    lives in o = n//256, survives on the partition axis), and emits
    per-ROW partial sums.  The host then applies the per-row
    standardization correction
        sin_part = sum_b inv_b * (S_sin[b] - mu_b * W_sin[b])
    where W_* are host-precomputed window-only DFT partials.  This removes
    the entire mean/std/broadcast serial chain from the device.
  - bf16 feeds the PE (validated: final rel err ~6e-6); PSUM stays f32.
  - All O(140) post-processing (softmax/KL/CE/argmax) happens on host in
    float32, mirroring the reference ops.
"""

import math

import numpy as np

B, T = 64, 4096
NCORES = 8
RPC = B // NCORES          # rows per core = 8
P = 128                    # SBUF partitions
F = (RPC * T) // P         # free dim = 256
NBINS = 140
N = B * T                  # 262144

_built = None


def _build_module():
    import concourse.bacc as bacc
    import concourse.bass as bass
    import concourse.mybir as mybir
    from concourse import tile

    f32 = mybir.dt.float32
    bf16 = mybir.dt.bfloat16
    ALU = mybir.AluOpType
    AX = mybir.AxisListType

    nc = bacc.Bacc(target_bir_lowering=False)

    x_d = nc.dram_tensor("x", [P, F], f32, kind="ExternalInput")
    y_d = nc.dram_tensor("y", [P, F], f32, kind="ExternalInput")
    xt_d = nc.dram_tensor("xt", [P, F], f32, kind="ExternalInput")
    wint_d = nc.dram_tensor("wint", [P, F], f32, kind="ExternalInput")
    innert_d = nc.dram_tensor("innert", [P, 4 * NBINS], bf16, kind="ExternalInput")
    outerA_d = nc.dram_tensor("outerA", [P, 2 * NBINS], f32, kind="ExternalInput")
    outerB_d = nc.dram_tensor("outerB", [P, 2 * NBINS], f32, kind="ExternalInput")
    rmask_d = nc.dram_tensor("rmask", [P, RPC], f32, kind="ExternalInput")
    rmaskb_d = nc.dram_tensor("rmaskb", [P, RPC], bf16, kind="ExternalInput")
    pear_d = nc.dram_tensor("pear", [RPC, 5], f32, kind="ExternalOutput")
    dft_d = nc.dram_tensor("dft", [RPC, 4 * NBINS], f32, kind="ExternalOutput")

    with tile.TileContext(nc) as tc:
        with (
            tc.tile_pool(name="sb", bufs=1) as pool,
            tc.tile_pool(name="ps", bufs=1, space=bass.MemorySpace.PSUM) as psum,
        ):
            # ---- input DMAs, spread across engine queues ----
            xt = pool.tile([P, F], f32)
            nc.sync.dma_start(xt[:], xt_d[:])
            wint = pool.tile([P, F], f32)
            nc.scalar.dma_start(wint[:], wint_d[:])
            innert = pool.tile([P, 4 * NBINS], bf16)
            nc.scalar.dma_start(innert[:], innert_d[:])
            outerA = pool.tile([P, 2 * NBINS], f32)
            nc.sync.dma_start(outerA[:], outerA_d[:])
            outerB = pool.tile([P, 2 * NBINS], f32)
            nc.gpsimd.dma_start(outerB[:], outerB_d[:])
            x = pool.tile([P, F], f32)
            nc.gpsimd.dma_start(x[:], x_d[:])
            y = pool.tile([P, F], f32)
            nc.gpsimd.dma_start(y[:], y_d[:])
            rmaskb = pool.tile([P, RPC], bf16)
            nc.scalar.dma_start(rmaskb[:], rmaskb_d[:])
            rmask = pool.tile([P, RPC], f32)
            nc.scalar.dma_start(rmask[:], rmask_d[:])

            # ---- DFT path (critical): raw windowed signal, per-row partials ----
            xw = pool.tile([P, F], bf16)
            nc.vector.tensor_mul(xw[:], xt[:], wint[:])
            U_ps = psum.tile([P, 2 * NBINS], f32)
            nc.tensor.matmul(U_ps[:], xw[:, 0:P], innert[:, 0:2 * NBINS],
                             start=True, stop=False)
            nc.tensor.matmul(U_ps[:], xw[:, P:2 * P], innert[:, 2 * NBINS:4 * NBINS],
                             start=False, stop=True)
            prod_s = pool.tile([P, 2 * NBINS], bf16)
            nc.vector.tensor_mul(prod_s[:], U_ps[:], outerA[:])
            prod_c = pool.tile([P, 2 * NBINS], bf16)
            nc.vector.tensor_mul(prod_c[:], U_ps[:], outerB[:])
            red_s = psum.tile([RPC, 2 * NBINS], f32)
            nc.tensor.matmul(red_s[:], rmaskb[:], prod_s[:])
            red_c = psum.tile([RPC, 2 * NBINS], f32)
            nc.tensor.matmul(red_c[:], rmaskb[:], prod_c[:])
            red_sb = pool.tile([RPC, 4 * NBINS], f32)
            nc.scalar.copy(red_sb[:, 0:2 * NBINS], red_s[:])
            nc.scalar.copy(red_sb[:, 2 * NBINS:4 * NBINS], red_c[:])
            nc.sync.dma_start(dft_d[:], red_sb[:])

            # ---- Pearson raw stats (parallel): [sx, sy, sxy, sx2, sy2] ----
            stats = pool.tile([P, 5], f32)
            nc.vector.reduce_sum(out=stats[:, 0:1], in_=x[:], axis=AX.X, op=ALU.add)
            nc.vector.reduce_sum(out=stats[:, 1:2], in_=y[:], axis=AX.X, op=ALU.add)
            xy = pool.tile([P, F], f32)
            nc.gpsimd.tensor_mul(xy[:], x[:], y[:])
            nc.vector.reduce_sum(out=stats[:, 2:3], in_=xy[:], axis=AX.X, op=ALU.add)
            xx = pool.tile([P, F], f32)
            nc.gpsimd.tensor_mul(xx[:], x[:], x[:])
            nc.vector.reduce_sum(out=stats[:, 3:4], in_=xx[:], axis=AX.X, op=ALU.add)
            yy = pool.tile([P, F], f32)
            nc.gpsimd.tensor_mul(yy[:], y[:], y[:])
            nc.vector.reduce_sum(out=stats[:, 4:5], in_=yy[:], axis=AX.X, op=ALU.add)
            pear_ps = psum.tile([RPC, 5], f32)
            nc.tensor.matmul(pear_ps[:], rmask[:], stats[:])
            pear_sb = pool.tile([RPC, 5], f32)
            nc.scalar.copy(pear_sb[:], pear_ps[:])
            nc.sync.dma_start(pear_d[:], pear_sb[:])

    nc.compile()
    return nc


def _tables(frame_rate: int):
    """Host-precomputed constant tables (depend only on frame_rate)."""
    import ml_dtypes

    nbf = ml_dtypes.bfloat16
    bpm = np.arange(40.0, 180.0, dtype=np.float32)
    k32 = (bpm / np.float32(60.0)) / (np.float32(frame_rate) / np.float32(N))
    theta = k32.astype(np.float64) * (2.0 * math.pi) / N       # [140]

    ov = np.arange(NCORES * P, dtype=np.float64)               # o = n // 256
    sinO = np.sin(theta[None, :] * F * ov[:, None])            # [1024, 140] f64
    cosO = np.cos(theta[None, :] * F * ov[:, None])
    rv = np.arange(F, dtype=np.float64)                        # r = n % 256
    sinI = np.sin(theta[None, :] * rv[:, None])                # [256, 140] f64
    cosI = np.cos(theta[None, :] * rv[:, None])

    # PE rhs for the r-contraction: per r-half h, [sinI_h | cosI_h]  (bf16)
    innert = np.concatenate(
        [sinI[0:P], cosI[0:P], sinI[P:2 * P], cosI[P:2 * P]], axis=1
    ).astype(nbf)                                              # [128, 560]

    # combine tables, per core (o rows): A = [cosO | sinO], B = [sinO | cosO]
    outerA, outerB = [], []
    for c in range(NCORES):
        sl = slice(c * P, (c + 1) * P)
        outerA.append(np.ascontiguousarray(np.concatenate(
            [cosO[sl], sinO[sl]], axis=1).astype(np.float32)))
        outerB.append(np.ascontiguousarray(np.concatenate(
            [sinO[sl], cosO[sl]], axis=1).astype(np.float32)))

    # transposed-layout hann window: wint[p, h*128+o_local] = w[256*o + 128*h + p]
    win = np.hanning(N).astype(np.float32)
    win_t = win.reshape(NCORES, P, 2, P).transpose(0, 3, 2, 1).reshape(NCORES, P, F)
    win_t = np.ascontiguousarray(win_t)

    # per-row window-only DFT partials (f64) for the host-side correction
    win2 = win.reshape(NCORES * P, F).astype(np.float64)
    W_sin = np.zeros((B, NBINS)); W_cos = np.zeros((B, NBINS))
    for b in range(B):
        sl = slice(b * 16, (b + 1) * 16)
        A = win2[sl] @ cosI                                    # [16, 140]
        Bm = win2[sl] @ sinI
        W_sin[b] = (sinO[sl] * A + cosO[sl] * Bm).sum(0)
        W_cos[b] = (cosO[sl] * A - sinO[sl] * Bm).sum(0)

    rmask = np.zeros((P, RPC), dtype=np.float32)
    for b in range(RPC):
        rmask[b * (P // RPC):(b + 1) * (P // RPC), b] = 1.0
    rmaskb = rmask.astype(nbf)

    return innert, outerA, outerB, win_t, W_sin, W_cos, rmask, rmaskb


_tables_cache = {}


def _make_in_maps(preds, targs, frame_rate):
    if frame_rate not in _tables_cache:
        _tables_cache[frame_rate] = _tables(frame_rate)
    innert, outerA, outerB, win_t, _, _, rmask, rmaskb = _tables_cache[frame_rate]
    in_maps = []
    for c in range(NCORES):
        xc = preds[c * RPC:(c + 1) * RPC].reshape(P, F)
        # transposed layout: xt[p, h*128+o] = x_flat[256*o + 128*h + p]
        xtc = np.ascontiguousarray(
            xc.reshape(P, 2, P).transpose(2, 1, 0).reshape(P, F))
        in_maps.append({
            "x": xc,
            "y": targs[c * RPC:(c + 1) * RPC].reshape(P, F),
            "xt": xtc,
            "wint": win_t[c],
            "innert": innert,
            "outerA": outerA[c],
            "outerB": outerB[c],
            "rmask": rmask,
            "rmaskb": rmaskb,
        })
    return in_maps


def _finish(results, avg_hr, a, b, frame_rate):
    _, _, _, _, W_sin, W_cos, _, _ = _tables_cache[frame_rate]

    # ---- Pearson from raw sums ----
    sums = np.concatenate([results[c]["pear"] for c in range(NCORES)], axis=0)
    sums = sums.astype(np.float32)                              # [64, 5]
    sum_x, sum_y, sum_xy, sum_x2, sum_y2 = (sums[:, i] for i in range(5))
    Nt = np.float32(T)
    pearson = (Nt * sum_xy - sum_x * sum_y) / np.sqrt(
        (Nt * sum_x2 - sum_x ** 2) * (Nt * sum_y2 - sum_y ** 2))
    loss_rppg = np.float32(np.mean(np.float32(1.0) - pearson, dtype=np.float32))

    # ---- spectrum: per-row raw partials + standardization correction ----
    d = np.concatenate([results[c]["dft"] for c in range(NCORES)], axis=0)
    d = d.astype(np.float64)                                    # [64, 560]
    S_sin = d[:, 0:NBINS] + d[:, NBINS:2 * NBINS]
    S_cos = d[:, 3 * NBINS:4 * NBINS] - d[:, 2 * NBINS:3 * NBINS]
    mu = (sum_x / Nt).astype(np.float64)
    ssq = sum_x2.astype(np.float64) - sum_x.astype(np.float64) * mu
    inv = 1.0 / np.sqrt(ssq / (T - 1))
    sin_part = (inv[:, None] * (S_sin - mu[:, None] * W_sin)).sum(0)
    cos_part = (inv[:, None] * (S_cos - mu[:, None] * W_cos)).sum(0)
    sin_part = sin_part.astype(np.float32)
    cos_part = cos_part.astype(np.float32)

    ca = sin_part ** 2 + cos_part ** 2
    ca = (ca / np.sum(ca)).astype(np.float32)

    t_idx = avg_hr - 40
    i = np.arange(NBINS, dtype=np.float64)
    td = np.exp(-(i - t_idx) ** 2 / 2.0) / math.sqrt(2.0 * math.pi)
    td = np.maximum(td, 1e-15).astype(np.float32)

    m = np.max(ca)
    e = np.exp(ca - m)
    freq = (e / np.sum(e)).astype(np.float32)
    loss_kl = np.float32(np.sum(td * (np.log(td) - np.log(freq))) / np.float32(140.0))

    loss_ce = np.float32(np.log(np.sum(np.exp(ca - m))) + m - ca[t_idx])
    mae_hr = np.float32(abs(float(t_idx) - float(np.argmax(ca))))

    total = np.float32(a) * loss_rppg + np.float32(b) * (loss_ce + loss_kl)
    return (np.float32(total), np.float32(loss_rppg), np.float32(loss_kl),
            np.float32(loss_ce), np.float32(mae_hr))


def kernel(predictions, targets, avg_hr, frame_rate, a, b):
    from concourse.bass_utils import run_bass_kernel_spmd

    global _built
    if _built is None:
        _built = _build_module()

    preds = np.ascontiguousarray(predictions, dtype=np.float32)
    targs = np.ascontiguousarray(targets, dtype=np.float32)
    in_maps = _make_in_maps(preds, targs, int(frame_rate))
    res = run_bass_kernel_spmd(nc=_built, in_maps=in_maps,
                               core_ids=list(range(NCORES)))
    return _finish(res.results, int(avg_hr), int(a), int(b), int(frame_rate))


# revision 12
# speedup vs baseline: 1.6349x; 1.0782x over previous
"""Trainium2 Bass kernel for CombinedLabelDistributionLoss.

Strategy (8 NeuronCores, SPMD, no collectives):
  - Batch-parallel: core c owns rows [8c, 8c+8) of predictions/targets.
  - Pearson is computed from RAW-data sums (sum x, y, xy, x^2, y^2) on
    device -- standardization cancels algebraically -- finished on host.
  - The 140-bin DFT over the flattened standardized signal uses the
    angle-addition factorization  sin(theta*(256*o + r)) =
    sinO[o]cosI[r] + cosO[o]sinI[r].  The device DFTs the RAW windowed
    signal (x*hann), contracting over r first (so the row identity, which
    lives in o = n//256, survives on the partition axis), and emits
    per-ROW partial sums.  The host then applies the per-row
    standardization correction
        sin_part = sum_b inv_b * (S_sin[b] - mu_b * W_sin[b])
    where W_* are host-precomputed window-only DFT partials.  This removes
    the entire mean/std/broadcast serial chain from the device.
  - bf16 feeds the PE (validated: final rel err ~6e-6); PSUM stays f32.
  - All O(140) post-processing (softmax/KL/CE/argmax) happens on host in
    float32, mirroring the reference ops.
"""

import math

import numpy as np

B, T = 64, 4096
NCORES = 8
RPC = B // NCORES          # rows per core = 8
P = 128                    # SBUF partitions
F = (RPC * T) // P         # free dim = 256
NBINS = 140
N = B * T                  # 262144

_built = None


def _build_module():
    import concourse.bacc as bacc
    import concourse.bass as bass
    import concourse.mybir as mybir
    from concourse import tile

    f32 = mybir.dt.float32
    bf16 = mybir.dt.bfloat16
    ALU = mybir.AluOpType
    AX = mybir.AxisListType

    nc = bacc.Bacc(target_bir_lowering=False)

    x_d = nc.dram_tensor("x", [P, F], f32, kind="ExternalInput")
    y_d = nc.dram_tensor("y", [P, F], f32, kind="ExternalInput")
    xt_d = nc.dram_tensor("xt", [P, F], f32, kind="ExternalInput")
    wint_d = nc.dram_tensor("wint", [P, F], f32, kind="ExternalInput")
    innert_d = nc.dram_tensor("innert", [P, 4 * NBINS], bf16, kind="ExternalInput")
    outerA_d = nc.dram_tensor("outerA", [P, 2 * NBINS], f32, kind="ExternalInput")
    outerB_d = nc.dram_tensor("outerB", [P, 2 * NBINS], f32, kind="ExternalInput")
    rmask_d = nc.dram_tensor("rmask", [P, RPC], f32, kind="ExternalInput")
    rmaskb_d = nc.dram_tensor("rmaskb", [P, RPC], bf16, kind="ExternalInput")
    out_d = nc.dram_tensor("out", [RPC, 4 * NBINS + 5], f32, kind="ExternalOutput")

    with tile.TileContext(nc) as tc:
        with (
            tc.tile_pool(name="sb", bufs=1) as pool,
            tc.tile_pool(name="ps", bufs=1, space=bass.MemorySpace.PSUM) as psum,
        ):
            # ---- input DMAs, spread across engine queues, critical first ----
            xt = pool.tile([P, F], f32)
            nc.sync.dma_start(xt[:], xt_d[:])
            innert = pool.tile([P, 4 * NBINS], bf16)
            nc.sync.dma_start(innert[:], innert_d[:])
            wint = pool.tile([P, F], f32)
            nc.scalar.dma_start(wint[:], wint_d[:])
            outerA = pool.tile([P, 2 * NBINS], f32)
            nc.scalar.dma_start(outerA[:], outerA_d[:])
            outerB = pool.tile([P, 2 * NBINS], f32)
            nc.scalar.dma_start(outerB[:], outerB_d[:])
            x = pool.tile([P, F], f32)
            nc.gpsimd.dma_start(x[:], x_d[:])
            y = pool.tile([P, F], f32)
            nc.gpsimd.dma_start(y[:], y_d[:])
            rmaskb = pool.tile([P, RPC], bf16)
            nc.gpsimd.dma_start(rmaskb[:], rmaskb_d[:])
            rmask = pool.tile([P, RPC], f32)
            nc.gpsimd.dma_start(rmask[:], rmask_d[:])

            # ---- DFT path (critical): raw windowed signal, per-row partials ----
            xw = pool.tile([P, F], bf16)
            nc.vector.tensor_mul(xw[:], xt[:], wint[:])
            U_ps = psum.tile([P, 2 * NBINS], f32)
            nc.tensor.matmul(U_ps[:], xw[:, 0:P], innert[:, 0:2 * NBINS],
                             start=True, stop=False)
            nc.tensor.matmul(U_ps[:], xw[:, P:2 * P], innert[:, 2 * NBINS:4 * NBINS],
                             start=False, stop=True)
            prod_s = pool.tile([P, 2 * NBINS], bf16)
            nc.vector.tensor_mul(prod_s[:], U_ps[:], outerA[:])
            prod_c = pool.tile([P, 2 * NBINS], bf16)
            nc.vector.tensor_mul(prod_c[:], U_ps[:], outerB[:])
            red_s = psum.tile([RPC, 2 * NBINS], f32)
            nc.tensor.matmul(red_s[:], rmaskb[:], prod_s[:])
            red_c = psum.tile([RPC, 2 * NBINS], f32)
            nc.tensor.matmul(red_c[:], rmaskb[:], prod_c[:])
            red_sb = pool.tile([RPC, 4 * NBINS + 5], f32)
            nc.scalar.copy(red_sb[:, 0:2 * NBINS], red_s[:])
            nc.vector.tensor_copy(red_sb[:, 2 * NBINS:4 * NBINS], red_c[:])

            # ---- Pearson raw stats (parallel): [sx, sy, sxy, sx2, sy2] ----
            stats = pool.tile([P, 5], f32)
            xx = pool.tile([P, F], f32)
            nc.gpsimd.tensor_mul(xx[:], x[:], x[:])
            xy = pool.tile([P, F], f32)
            nc.gpsimd.tensor_mul(xy[:], x[:], y[:])
            yy = pool.tile([P, F], f32)
            nc.gpsimd.tensor_mul(yy[:], y[:], y[:])
            nc.vector.reduce_sum(out=stats[:, 0:1], in_=x[:], axis=AX.X, op=ALU.add)
            nc.vector.reduce_sum(out=stats[:, 3:4], in_=xx[:], axis=AX.X, op=ALU.add)
            nc.vector.reduce_sum(out=stats[:, 1:2], in_=y[:], axis=AX.X, op=ALU.add)
            nc.vector.reduce_sum(out=stats[:, 2:3], in_=xy[:], axis=AX.X, op=ALU.add)
            nc.vector.reduce_sum(out=stats[:, 4:5], in_=yy[:], axis=AX.X, op=ALU.add)
            pear_ps = psum.tile([RPC, 5], f32)
            nc.tensor.matmul(pear_ps[:], rmask[:], stats[:])
            nc.scalar.copy(red_sb[:, 4 * NBINS:4 * NBINS + 5], pear_ps[:])
            nc.sync.dma_start(out_d[:], red_sb[:])

    nc.compile()
    return nc


def _tables(frame_rate: int):
    """Host-precomputed constant tables (depend only on frame_rate)."""
    import ml_dtypes

    nbf = ml_dtypes.bfloat16
    bpm = np.arange(40.0, 180.0, dtype=np.float32)
    k32 = (bpm / np.float32(60.0)) / (np.float32(frame_rate) / np.float32(N))
    theta = k32.astype(np.float64) * (2.0 * math.pi) / N       # [140]

    ov = np.arange(NCORES * P, dtype=np.float64)               # o = n // 256
    sinO = np.sin(theta[None, :] * F * ov[:, None])            # [1024, 140] f64
    cosO = np.cos(theta[None, :] * F * ov[:, None])
    rv = np.arange(F, dtype=np.float64)                        # r = n % 256
    sinI = np.sin(theta[None, :] * rv[:, None])                # [256, 140] f64
    cosI = np.cos(theta[None, :] * rv[:, None])

    # PE rhs for the r-contraction: per r-half h, [sinI_h | cosI_h]  (bf16)
    innert = np.concatenate(
        [sinI[0:P], cosI[0:P], sinI[P:2 * P], cosI[P:2 * P]], axis=1
    ).astype(nbf)                                              # [128, 560]

    # combine tables, per core (o rows): A = [cosO | sinO], B = [sinO | cosO]
    outerA, outerB = [], []
    for c in range(NCORES):
        sl = slice(c * P, (c + 1) * P)
        outerA.append(np.ascontiguousarray(np.concatenate(
            [cosO[sl], sinO[sl]], axis=1).astype(np.float32)))
        outerB.append(np.ascontiguousarray(np.concatenate(
            [sinO[sl], cosO[sl]], axis=1).astype(np.float32)))

    # transposed-layout hann window: wint[p, h*128+o_local] = w[256*o + 128*h + p]
    win = np.hanning(N).astype(np.float32)
    win_t = win.reshape(NCORES, P, 2, P).transpose(0, 3, 2, 1).reshape(NCORES, P, F)
    win_t = np.ascontiguousarray(win_t)

    # per-row window-only DFT partials (f64) for the host-side correction
    win2 = win.reshape(NCORES * P, F).astype(np.float64)
    W_sin = np.zeros((B, NBINS)); W_cos = np.zeros((B, NBINS))
    for b in range(B):
        sl = slice(b * 16, (b + 1) * 16)
        A = win2[sl] @ cosI                                    # [16, 140]
        Bm = win2[sl] @ sinI
        W_sin[b] = (sinO[sl] * A + cosO[sl] * Bm).sum(0)
        W_cos[b] = (cosO[sl] * A - sinO[sl] * Bm).sum(0)

    rmask = np.zeros((P, RPC), dtype=np.float32)
    for b in range(RPC):
        rmask[b * (P // RPC):(b + 1) * (P // RPC), b] = 1.0
    rmaskb = rmask.astype(nbf)

    return innert, outerA, outerB, win_t, W_sin, W_cos, rmask, rmaskb


_tables_cache = {}


def _make_in_maps(preds, targs, frame_rate):
    if frame_rate not in _tables_cache:
        _tables_cache[frame_rate] = _tables(frame_rate)
    innert, outerA, outerB, win_t, _, _, rmask, rmaskb = _tables_cache[frame_rate]
    in_maps = []
    for c in range(NCORES):
        xc = preds[c * RPC:(c + 1) * RPC].reshape(P, F)
        # transposed layout: xt[p, h*128+o] = x_flat[256*o + 128*h + p]
        xtc = np.ascontiguousarray(
            xc.reshape(P, 2, P).transpose(2, 1, 0).reshape(P, F))
        in_maps.append({
            "x": xc,
            "y": targs[c * RPC:(c + 1) * RPC].reshape(P, F),
            "xt": xtc,
            "wint": win_t[c],
            "innert": innert,
            "outerA": outerA[c],
            "outerB": outerB[c],
            "rmask": rmask,
            "rmaskb": rmaskb,
        })
    return in_maps


def _finish(results, avg_hr, a, b, frame_rate):
    _, _, _, _, W_sin, W_cos, _, _ = _tables_cache[frame_rate]

    # ---- Pearson from raw sums ----
    outs = np.concatenate([results[c]["out"] for c in range(NCORES)], axis=0)
    sums = outs[:, 4 * NBINS:4 * NBINS + 5].astype(np.float32)  # [64, 5]
    sum_x, sum_y, sum_xy, sum_x2, sum_y2 = (sums[:, i] for i in range(5))
    Nt = np.float32(T)
    pearson = (Nt * sum_xy - sum_x * sum_y) / np.sqrt(
        (Nt * sum_x2 - sum_x ** 2) * (Nt * sum_y2 - sum_y ** 2))
    loss_rppg = np.float32(np.mean(np.float32(1.0) - pearson, dtype=np.float32))

    # ---- spectrum: per-row raw partials + standardization correction ----
    d = outs[:, 0:4 * NBINS].astype(np.float64)                 # [64, 560]
    S_sin = d[:, 0:NBINS] + d[:, NBINS:2 * NBINS]
    S_cos = d[:, 3 * NBINS:4 * NBINS] - d[:, 2 * NBINS:3 * NBINS]
    mu = (sum_x / Nt).astype(np.float64)
    ssq = sum_x2.astype(np.float64) - sum_x.astype(np.float64) * mu
    inv = 1.0 / np.sqrt(ssq / (T - 1))
    sin_part = (inv[:, None] * (S_sin - mu[:, None] * W_sin)).sum(0)
    cos_part = (inv[:, None] * (S_cos - mu[:, None] * W_cos)).sum(0)
    sin_part = sin_part.astype(np.float32)
    cos_part = cos_part.astype(np.float32)

    ca = sin_part ** 2 + cos_part ** 2
    ca = (ca / np.sum(ca)).astype(np.float32)

    t_idx = avg_hr - 40
    i = np.arange(NBINS, dtype=np.float64)
    td = np.exp(-(i - t_idx) ** 2 / 2.0) / math.sqrt(2.0 * math.pi)
    td = np.maximum(td, 1e-15).astype(np.float32)

    m = np.max(ca)
    e = np.exp(ca - m)
    freq = (e / np.sum(e)).astype(np.float32)
    loss_kl = np.float32(np.sum(td * (np.log(td) - np.log(freq))) / np.float32(140.0))

    loss_ce = np.float32(np.log(np.sum(np.exp(ca - m))) + m - ca[t_idx])
    mae_hr = np.float32(abs(float(t_idx) - float(np.argmax(ca))))

    total = np.float32(a) * loss_rppg + np.float32(b) * (loss_ce + loss_kl)
    return (np.float32(total), np.float32(loss_rppg), np.float32(loss_kl),
            np.float32(loss_ce), np.float32(mae_hr))


def kernel(predictions, targets, avg_hr, frame_rate, a, b):
    from concourse.bass_utils import run_bass_kernel_spmd

    global _built
    if _built is None:
        _built = _build_module()

    preds = np.ascontiguousarray(predictions, dtype=np.float32)
    targs = np.ascontiguousarray(targets, dtype=np.float32)
    in_maps = _make_in_maps(preds, targs, int(frame_rate))
    res = run_bass_kernel_spmd(nc=_built, in_maps=in_maps,
                               core_ids=list(range(NCORES)))
    return _finish(res.results, int(avg_hr), int(a), int(b), int(frame_rate))
